# revision 9
# baseline (speedup 1.0000x reference)
"""Multi-head attention (b=2, n=2048, 16 heads x 64, RoPE) on 8 TRN2 NeuronCores.

Sharding: core = 4*b + g handles batch b (0..1) and head-group g (0..3, i.e.
heads 4g..4g+3).  Each core computes its partial output projection
out_partial[b] = O_g @ Wo[256g:256g+256, :]; the host sums the 4 partials per
batch and adds the bias.

Device layout (per core, everything transposed so the contraction dim sits on
SBUF partitions):
  xT   [1024, 2048]  x[b].T                        (host pre-transposed)
  wq/wqs/wk/wks/wv [1024, 256], wo [256, 1024]     (wqs/wks: RoPE-swapped cols)
  cosT/sinT [128, 2048]                            (RoPE tables, head-pair rows)
Pipeline: QK projections (+swapped twins) -> RoPE combine on DVE -> V
projection -> per head-pair: S^T = K Q^T on PE (row-group concurrent pairs),
exp on ACT, O'^T = [V|1]^T P^T on PE (ones column yields softmax denominators)
-> normalize -> output projection.
All matmuls run as float32r (TF32-like, 1 cyc/row).
"""

import numpy as np

HEADS = 16
DH = 64
THETA = 10000.0
B, N, DIM = 2, 2048, 1024
GCOLS = 4 * DH  # 256 columns per head-group
P = 128
NQC = 512  # nq chunk (psum bank)
NCH = N // NQC  # 4
KT = DIM // P  # 8 contraction subtiles for projections
NKT = N // P  # 16 nk tiles

_cache = {}


def _rope_tables():
    """cosT/sinT [128, 2048] in transposed (d, n) layout, head-pair rows.
    Replicates reference fp32 arithmetic."""
    d = np.float32(DH)
    inv_freq = np.float32(1.0) / (
        np.float32(THETA) ** (np.arange(0, DH, 2, dtype=np.float32) / d)
    )  # [32]
    ang = np.arange(N, dtype=np.float32)[:, None] * inv_freq[None, :]  # [n, 32] fp32
    ang = np.repeat(ang, 2, axis=-1)  # [n, 64]
    cos = np.cos(ang).astype(np.float32).T  # [64, n]
    sin = np.sin(ang).astype(np.float32).T
    cosT = np.concatenate([cos, cos], axis=0)  # [128, n]
    sinT = np.concatenate([sin, sin], axis=0)
    return np.ascontiguousarray(cosT), np.ascontiguousarray(sinT)


def _swap_cols(w):
    """RoPE rotate-half as a column permutation: col 2i <- -col 2i+1, col 2i+1 <- col 2i."""
    ws = np.empty_like(w)
    ws[:, 0::2] = -w[:, 1::2]
    ws[:, 1::2] = w[:, 0::2]
    return ws


def _build(debug=False):
    import concourse.bacc as bacc
    import concourse.tile as tile
    import concourse.mybir as mybir
    from contextlib import ExitStack

    f32 = mybir.dt.float32
    f32r = mybir.dt.float32r
    Exp = mybir.ActivationFunctionType.Exp

    nc = bacc.Bacc("TRN2", target_bir_lowering=False, debug=False)

    xT = nc.dram_tensor("xT", [DIM, N], f32r, kind="ExternalInput")[:]
    wq = nc.dram_tensor("wq", [DIM, GCOLS], f32r, kind="ExternalInput")[:]
    wqs = nc.dram_tensor("wqs", [DIM, GCOLS], f32r, kind="ExternalInput")[:]
    wk = nc.dram_tensor("wk", [DIM, GCOLS], f32r, kind="ExternalInput")[:]
    wks = nc.dram_tensor("wks", [DIM, GCOLS], f32r, kind="ExternalInput")[:]
    wv = nc.dram_tensor("wv", [DIM, GCOLS], f32r, kind="ExternalInput")[:]
    wo = nc.dram_tensor("wo", [GCOLS, DIM], f32r, kind="ExternalInput")[:]
    cosT = nc.dram_tensor("cosT", [P, N], f32, kind="ExternalInput")[:]
    sinT = nc.dram_tensor("sinT", [P, N], f32, kind="ExternalInput")[:]
    out = nc.dram_tensor("out", [N, DIM], f32, kind="ExternalOutput")[:]
    if debug:
        dbg_qt = nc.dram_tensor("dbg_qt", [P, 2, N], f32, kind="ExternalOutput")[:]
        dbg_kt = nc.dram_tensor("dbg_kt", [P, 2, N], f32, kind="ExternalOutput")[:]
        dbg_v = nc.dram_tensor("dbg_v", [P, NKT, 4, DH + 1], f32, kind="ExternalOutput")[:]
        dbg_e = nc.dram_tensor("dbg_e", [P, 2, NQC], f32, kind="ExternalOutput")[:]
        dbg_pso = nc.dram_tensor("dbg_pso", [DH + 1, NQC], f32, kind="ExternalOutput")[:]
        dbg_bc = nc.dram_tensor("dbg_bc", [DH, NQC], f32, kind="ExternalOutput")[:]
        dbg_ot = nc.dram_tensor("dbg_ot", [P, 2, N], f32, kind="ExternalOutput")[:]

    with tile.TileContext(nc) as tc, ExitStack() as ctx:
        persist = ctx.enter_context(tc.tile_pool(name="persist", bufs=1))
        qt_sb = persist.tile([P, 2, N], f32r, tag="qt")
        kt_sb = persist.tile([P, 2, N], f32r, tag="kt")
        v_sb = persist.tile([P, NKT, 4, DH + 1], f32r, tag="v")
        ot_sb = persist.tile([P, 2, N], f32r, tag="ot")

        with (
            tc.tile_pool(name="xtp", bufs=1) as xtp,
            tc.tile_pool(name="wvp", bufs=1) as wvp,
            tc.tile_pool(name="psA", bufs=4, space="PSUM") as psA,
        ):
            xt_sb = xtp.tile([P, KT, N], f32r, tag="xt")
            for k in range(KT):
                nc.sync.dma_start(xt_sb[:, k, :], xT[k * P:(k + 1) * P, :])
            wv_sb = wvp.tile([P, KT, GCOLS], f32r, tag="wv")
            nc.sync.dma_start(wv_sb, wv.rearrange("(ko p) c -> p ko c", p=P))

            with (
                tc.tile_pool(name="trig", bufs=1) as trig,
                tc.tile_pool(name="wst", bufs=3) as wst,
                tc.tile_pool(name="ropetmp", bufs=4) as ropetmp,
            ):
                cos_sb = trig.tile([P, N], f32, tag="cos")
                sin_sb = trig.tile([P, N], f32, tag="sin")
                nc.sync.dma_start(cos_sb, cosT)
                nc.sync.dma_start(sin_sb, sinT)

                # ---- Phase 1: Q/K projections (plus swapped twins) + RoPE
                for (w_d, ws_d, dst) in ((wq, wqs, qt_sb), (wk, wks, kt_sb)):
                    for m in range(2):  # head-pair (128 cols of the 256)
                        wt = wst.tile([P, KT, P], f32r, tag="wt")
                        wts = wst.tile([P, KT, P], f32r, tag="wt")
                        nc.sync.dma_start(
                            wt, w_d[:, m * P:(m + 1) * P].rearrange("(ko p) c -> p ko c", p=P))
                        nc.sync.dma_start(
                            wts, ws_d[:, m * P:(m + 1) * P].rearrange("(ko p) c -> p ko c", p=P))
                        for c in range(NCH):
                            cs = slice(c * NQC, (c + 1) * NQC)
                            ps_q = psA.tile([P, NQC], f32, tag="ps_proj")
                            ps_s = psA.tile([P, NQC], f32, tag="ps_proj")
                            for k in range(KT):
                                nc.tensor.matmul(ps_q, wt[:, k, :], xt_sb[:, k, cs],
                                                 start=(k == 0), stop=(k == KT - 1))
                            for k in range(KT):
                                nc.tensor.matmul(ps_s, wts[:, k, :], xt_sb[:, k, cs],
                                                 start=(k == 0), stop=(k == KT - 1))
                            t1 = ropetmp.tile([P, NQC], f32, tag="rt1")
                            t2 = ropetmp.tile([P, NQC], f32, tag="rt2")
                            nc.vector.tensor_mul(t1, ps_q, cos_sb[:, cs])
                            nc.vector.tensor_mul(t2, ps_s, sin_sb[:, cs])
                            nc.vector.tensor_add(dst[:, m, cs], t1, t2)

            # ---- Phase 2: V projection -> [nk, 4 heads, 65] with ones column
            ones_sb = wvp.tile([P, 1], f32, tag="ones")
            nc.vector.memset(ones_sb, 1.0)
            nc.vector.tensor_copy(
                out=v_sb[:, :, :, DH],
                in_=ones_sb[:, 0:1].to_broadcast((P, NKT, 4)))
            for t in range(NKT):
                ps_v = psA.tile([P, GCOLS], f32, tag="ps_v")
                for k in range(KT):
                    nc.tensor.matmul(ps_v, xt_sb[:, k, t * P:(t + 1) * P], wv_sb[:, k, :],
                                     start=(k == 0), stop=(k == KT - 1))
                nc.vector.tensor_copy(
                    out=v_sb[:, t, :, 0:DH],
                    in_=ps_v.rearrange("p (h d) -> p h d", d=DH))

        if debug:
            nc.sync.dma_start(dbg_qt, qt_sb.bitcast(f32))
            nc.sync.dma_start(dbg_kt, kt_sb.bitcast(f32))
            nc.sync.dma_start(dbg_v, v_sb.bitcast(f32))

        # ---- Phase 3: attention per head pair
        with (
            tc.tile_pool(name="wop", bufs=1) as wop,
            tc.tile_pool(name="outp", bufs=3) as outp,
        ):
            wo_sb = wop.tile([P, 2, DIM], f32r, tag="wo")
            nc.sync.dma_start(wo_sb, wo.rearrange("(ko p) c -> p ko c", p=P))

            with (
                tc.tile_pool(name="ep", bufs=3) as ep,
                tc.tile_pool(name="nrm", bufs=2) as nrm,
                tc.tile_pool(name="psB", bufs=2, space="PSUM") as psB,
            ):
                for hp in range(2):  # head pair: heads (2*hp, 2*hp+1) of this group
                    for c in range(NCH):
                        cs = slice(c * NQC, (c + 1) * NQC)
                        ps_oA = psB.tile([DH + 1, NQC], f32, tag="ps_oA")
                        ps_oB = psB.tile([DH + 1, NQC], f32, tag="ps_oB")
                        for t in range(NKT):
                            ts_ = slice(t * P, (t + 1) * P)
                            ps_t = psB.tile([P, 2, NQC], f32, tag="ps_t")
                            # S^T tiles for both heads; row groups 0-1 and 2-3 run
                            # concurrently in the PE array.
                            nc.tensor.matmul(ps_t[:, 0, :], kt_sb[0:DH, hp, ts_],
                                             qt_sb[0:DH, hp, cs], start=True, stop=True)
                            nc.tensor.matmul(ps_t[:, 1, :], kt_sb[DH:P, hp, ts_],
                                             qt_sb[DH:P, hp, cs], start=True, stop=True)
                            e_t = ep.tile([P, 2, NQC], f32r, tag="e")
                            nc.scalar.activation(e_t, ps_t, Exp, scale=0.125)
                            if debug and hp == 0 and c == 0 and t == 0:
                                nc.sync.dma_start(dbg_e, e_t.bitcast(f32))
                            nc.tensor.matmul(ps_oA, v_sb[:, t, 2 * hp, :], e_t[:, 0, :],
                                             start=(t == 0), stop=(t == NKT - 1))
                            nc.tensor.matmul(ps_oB, v_sb[:, t, 2 * hp + 1, :], e_t[:, 1, :],
                                             start=(t == 0), stop=(t == NKT - 1))
                        # normalize: ot[d, nq] = O'[d, nq] / den[nq]
                        for idx, ps_o in ((0, ps_oA), (1, ps_oB)):
                            rec = nrm.tile([P, NQC], f32, tag="rec")
                            bc = nrm.tile([DH, NQC], f32, tag="bc")
                            nc.vector.reciprocal(rec[0:1, :], ps_o[DH:DH + 1, :])
                            nc.gpsimd.partition_broadcast(bc, rec[0:1, :])
                            if debug and hp == 0 and c == 0 and idx == 0:
                                ddt = nrm.tile([DH + 1, NQC], f32, tag="ddt")
                                nc.vector.tensor_copy(out=ddt, in_=ps_o)
                                nc.sync.dma_start(dbg_pso, ddt)
                                nc.sync.dma_start(dbg_bc, bc)
                            nc.vector.tensor_mul(
                                ot_sb[idx * DH:(idx + 1) * DH, hp, cs],
                                ps_o[0:DH, :], bc)

            if debug:
                nc.sync.dma_start(dbg_ot, ot_sb.bitcast(f32))

            # ---- Phase 4: output projection out = Ot.T @ Wo
            with tc.tile_pool(name="psC", bufs=3, space="PSUM") as psC:
                for nt in range(NKT):
                    for oc in range(2):
                        ps_w = psC.tile([P, NQC], f32, tag="ps_w")
                        for k in range(2):
                            nc.tensor.matmul(ps_w, ot_sb[:, k, nt * P:(nt + 1) * P],
                                             wo_sb[:, k, oc * NQC:(oc + 1) * NQC],
                                             start=(k == 0), stop=(k == 1))
                        o_t = outp.tile([P, NQC], f32, tag="o")
                        nc.vector.tensor_copy(out=o_t, in_=ps_w)
                        nc.sync.dma_start(
                            out[nt * P:(nt + 1) * P, oc * NQC:(oc + 1) * NQC], o_t)
    nc.compile()
    return nc


def _prep_inputs(x, Wq, Wkv, Wo):
    """Host-side sharding: returns in_maps for the 8 cores."""
    cosT, sinT = _rope_tables()
    xTs = [np.ascontiguousarray(x[b].T) for b in range(B)]
    in_maps = []
    for core in range(8):
        b, g = divmod(core, 4)
        cols = slice(g * GCOLS, (g + 1) * GCOLS)
        wq_g = np.ascontiguousarray(Wq[:, cols])
        wk_g = np.ascontiguousarray(Wkv[:, g * GCOLS:(g + 1) * GCOLS])
        wv_g = np.ascontiguousarray(Wkv[:, DIM + g * GCOLS:DIM + (g + 1) * GCOLS])
        wo_g = np.ascontiguousarray(Wo[cols, :])
        in_maps.append({
            "xT": xTs[b],
            "wq": wq_g, "wqs": _swap_cols(wq_g),
            "wk": wk_g, "wks": _swap_cols(wk_g),
            "wv": wv_g, "wo": wo_g,
            "cosT": cosT, "sinT": sinT,
        })
    return in_maps


def kernel(x, Wq, Wkv, Wo, bo):
    from concourse.bass_utils import run_bass_kernel_spmd

    x = np.asarray(x, dtype=np.float32)
    Wq = np.asarray(Wq, dtype=np.float32)
    Wkv = np.asarray(Wkv, dtype=np.float32)
    Wo = np.asarray(Wo, dtype=np.float32)
    bo = np.asarray(bo, dtype=np.float32)

    if "nc" not in _cache:
        _cache["nc"] = _build()
    nc = _cache["nc"]

    in_maps = _prep_inputs(x, Wq, Wkv, Wo)
    import os
    res = run_bass_kernel_spmd(nc, in_maps, core_ids=list(range(8)))
    _cache["last_results"] = res

    full = np.zeros((B, N, DIM), dtype=np.float32)
    for core in range(8):
        b = core // 4
        full[b] += res.results[core]["out"]
    full += bo[None, None, :]
    return full
